# revision 34
# baseline (speedup 1.0000x reference)
"""Trainium2 Bass kernel for ControlLoRACrossAttnProcessor (v2).

Batch x head-group sharding over 8 NeuronCores: core c owns batch c//4
and heads 2*(c%4), 2*(c%4)+1.  Each core computes both heads' full
attention over its batch and a partial output projection (contraction
over its 256 Wo rows); the host sums 4 partials per batch.  The rank-4
LoRA path is sharded by global sequence rows (core c owns rows
c*512:(c+1)*512) and emitted at the START of the program so it doubles
as PE warm-up; the output bias rides the LoRA partial (added exactly
once per row).

All matmuls are bf16 (hosts casts inputs); PSUM accumulates fp32.
Attention uses the transposed-scores layout: scores^T tiles
[k=128, q=512] so softmax's exp rides ScalarE.  Softmax denominators
avoid the PE entirely: a DVE tree-add collapses the 16 exp tiles per
(head, strip) to [128, 512], GpSimd partition_all_reduce produces the
per-q sums broadcast over all partitions, DVE reciprocal + tensor_mul
normalize the PV accumulator before the (per-q linear) output
projection.
"""

import sys

for _p in ("/opt/trn_rl_repo", "/root/.axon_site"):
    if _p not in sys.path:
        sys.path.insert(0, _p)

import numpy as np
import ml_dtypes

import concourse.bass as bass  # noqa: E402
import concourse.mybir as mybir  # noqa: E402
import concourse.bass_isa as bass_isa  # noqa: E402
from concourse import bacc  # noqa: E402
from concourse.bass_utils import run_bass_kernel_spmd  # noqa: E402
from concourse.tile import TileContext  # noqa: E402
from concourse.masks import make_identity  # noqa: E402

dt = mybir.dt

B, S, D = 2, 2048, 1024
H = 8
HD = 128
RANK = 4
N_CORES = 8
SG = B * S
ROWS_PER_CORE = SG // N_CORES      # 512 lora rows per core
NSTRIP = S // 512                  # 4 query strips of 512
NKT = S // 128                     # 16 key tiles of 128
HEADS_PER_CORE = 2
INV_SQRT_HD = 1.0 / np.sqrt(np.float32(HD))

F32 = dt.float32
BF16 = dt.bfloat16

_CACHE = {}


def build_program():
    if "nc" in _CACHE:
        return _CACHE["nc"]

    nc = bacc.Bacc("TRN2", target_bir_lowering=False, debug=False,
                   num_devices=N_CORES)

    # xsh/ct/w tensors are pre-shuffled on the host so each DMA is a plain
    # contiguous 2D copy (sbuf image layout), minimizing sync-engine
    # descriptor-generation time per dma_start.
    xsh = nc.declare_dram_parameter("xsh", [NSTRIP * 128, 8 * 512], BF16,
                                    isOutput=False)
    wqT = nc.declare_dram_parameter("wqT", [2 * 128, 8 * HD], BF16,
                                    isOutput=False)
    wkT = nc.declare_dram_parameter("wkT", [2 * 128, 8 * HD], BF16,
                                    isOutput=False)
    wvT = nc.declare_dram_parameter("wvT", [2 * 128, 8 * HD], BF16,
                                    isOutput=False)
    woT = nc.declare_dram_parameter("woT", [256, D], BF16, isOutput=False)
    cT = nc.declare_dram_parameter("cT", [128, 8 * ROWS_PER_CORE], BF16,
                                   isOutput=False)
    ldT = nc.declare_dram_parameter("ldT", [128, 8 * RANK], BF16,
                                    isOutput=False)
    luT = nc.declare_dram_parameter("luT", [RANK, D], BF16, isOutput=False)
    bo = nc.declare_dram_parameter("bo", [1, D], F32, isOutput=False)
    out = nc.declare_dram_parameter("out", [S, D], BF16, isOutput=True)
    lora_out = nc.declare_dram_parameter("lora_out", [128, 4 * D],
                                         BF16, isOutput=True)

    with TileContext(nc) as tc:
        with tc.tile_pool(name="const", bufs=1) as constp, \
             tc.tile_pool(name="wts", bufs=1) as wts, \
             tc.tile_pool(name="xs", bufs=1) as xsp, \
             tc.tile_pool(name="qkv", bufs=1) as qkvp, \
             tc.tile_pool(name="stage", bufs=2) as stagep, \
             tc.tile_pool(name="es", bufs=12) as esp, \
             tc.tile_pool(name="sp", bufs=8) as spp, \
             tc.tile_pool(name="small", bufs=2) as smallp, \
             tc.tile_pool(name="outp", bufs=2) as outp, \
             tc.tile_pool(name="sc_ps", bufs=2, space="PSUM") as sc_ps, \
             tc.tile_pool(name="at_ps", bufs=2, space="PSUM") as at_ps, \
             tc.tile_pool(name="op_ps", bufs=2, space="PSUM") as op_ps:

            # ---- constants ----
            ident = constp.tile([128, 128], BF16, tag="ident")
            make_identity(nc, ident[:])
            dummy = constp.tile([128, 512], BF16, tag="dummy")
            nc.vector.memset(dummy[:], 0.0)
            # rank-1 all-ones lhsT: one matmul = partition-sum + broadcast
            ones_sq = constp.tile([128, 128], BF16, tag="ones_sq")
            nc.vector.memset(ones_sq[:], 1.0)

            # ---- DMA loads, ordered for earliest compute start ----
            ld_sb = wts.tile([128, 8 * RANK], BF16, tag="ld")
            lu_sb = wts.tile([RANK, D], BF16, tag="lu")
            bo_sb = wts.tile([1, D], F32, tag="bo")
            bo_bc = wts.tile([128, D], F32, tag="bobc")
            ct_sb = wts.tile([128, 8 * ROWS_PER_CORE], BF16, tag="ct")
            nc.sync.dma_start(out=ld_sb[:], in_=ldT[:])
            nc.sync.dma_start(out=lu_sb[:], in_=luT[:])
            nc.sync.dma_start(out=bo_sb[:], in_=bo[:])

            w_sb = {}   # (which, head) -> [128, 8*128] tile
            for h in range(HEADS_PER_CORE):
                for nm in ("q", "k", "v"):
                    t = wts.tile([128, 8 * HD], BF16, tag=f"w{nm}{h}",
                                 name=f"w{nm}{h}")
                    w_sb[(nm, h)] = t

            xs = []
            for s in range(NSTRIP):
                t = xsp.tile([128, 8 * 512], BF16, tag=f"xs{s}",
                             name=f"xs{s}")
                xs.append(t)

            def load_xs(s):
                nc.sync.dma_start(out=xs[s][:],
                                  in_=xsh[s * 128:(s + 1) * 128, :])

            def load_w(h):
                for nm, src in (("q", wqT), ("k", wkT), ("v", wvT)):
                    nc.sync.dma_start(
                        out=w_sb[(nm, h)][:],
                        in_=src[h * 128:(h + 1) * 128, :])

            load_w(0)
            # strip 0 in two halves so the first proj can start on chunks 0-3
            nc.sync.dma_start(out=xs[0][:, 0:2048], in_=xsh[0:128, 0:2048])
            nc.sync.dma_start(out=xs[0][:, 2048:4096],
                              in_=xsh[0:128, 2048:4096])
            load_xs(1)
            nc.sync.dma_start(out=ct_sb[:], in_=cT[:])
            load_xs(2)
            load_xs(3)
            load_w(1)

            wo_sb = [wts.tile([HD, D], BF16, tag=f"wo{h}", name=f"wo{h}")
                     for h in range(HEADS_PER_CORE)]
            for h in range(HEADS_PER_CORE):
                nc.sync.dma_start(out=wo_sb[h][:],
                                  in_=woT[h * HD:(h + 1) * HD, :])

            # ---- PE warmup on the zero tile while first DMAs land ----
            for _wu in range(22):
                wu_ps = op_ps.tile([128, 512], F32, tag="op", name="wu_ps")
                nc.tensor.matmul(wu_ps[:], dummy[:, 0:128], dummy[:],
                                 start=True, stop=True)

            nc.gpsimd.partition_broadcast(bo_bc[:], bo_sb[:])

            # ---- LoRA path (also serves as further PE warmup) ----
            def emit_lora():
                dn_ps = op_ps.tile([128, ROWS_PER_CORE], F32, tag="op",
                                   name="dn_ps")
                for d in range(8):
                    nc.tensor.matmul(
                        dn_ps[0:RANK, :],
                        ld_sb[:, d * RANK:(d + 1) * RANK],
                        ct_sb[:, d * ROWS_PER_CORE:(d + 1) * ROWS_PER_CORE],
                        start=(d == 0), stop=(d == 7))
                dn_sb = smallp.tile([RANK, ROWS_PER_CORE], BF16, tag="dn",
                                    name="dn_sb")
                nc.scalar.copy(dn_sb[:], dn_ps[0:RANK, :])
                lo_sb = outp.tile([128, 4 * D], BF16, tag="lo", bufs=1,
                                  name="lo_sb")
                for j in range(ROWS_PER_CORE // 128):
                    for g in range(2):
                        up_ps = op_ps.tile([128, 512], F32, tag="op",
                                           name="up_ps")
                        nc.tensor.matmul(
                            up_ps[:],
                            dn_sb[:, j * 128:(j + 1) * 128],
                            lu_sb[:, g * 512:(g + 1) * 512],
                            start=True, stop=True)
                        nc.vector.tensor_add(
                            lo_sb[:, j * D + g * 512:j * D + (g + 1) * 512],
                            up_ps[:], bo_bc[:, g * 512:(g + 1) * 512])
                nc.sync.dma_start(out=lora_out[:], in_=lo_sb[:])

            # ---- QKV projection for one (head, strip) ----
            qt = [qkvp.tile([HD, S], BF16, tag=f"qt{h}", name=f"qt{h}")
                  for h in range(HEADS_PER_CORE)]
            kt = [qkvp.tile([HD, S], BF16, tag=f"kt{h}", name=f"kt{h}")
                  for h in range(HEADS_PER_CORE)]
            v_sb = [qkvp.tile([128, S], BF16, tag=f"v{h}", name=f"v{h}")
                    for h in range(HEADS_PER_CORE)]

            def qkv_strip(h, s, ce):
                sl = slice(s * 512, (s + 1) * 512)

                def proj(nm):
                    ps = op_ps.tile([128, 512], F32, tag="op", name="pj_ps")
                    for d in range(8):
                        nc.tensor.matmul(
                            ps[:],
                            w_sb[(nm, h)][:, d * HD:(d + 1) * HD],
                            xs[s][:, d * 512:(d + 1) * 512],
                            start=(d == 0), stop=(d == 7),
                            skip_group_check=True)
                    return ps

                ps_v = proj("v")
                vt_stage = stagep.tile([HD, 512], BF16, tag="vst",
                                       name="vt_stage")
                ce(vt_stage[:], ps_v[:])
                tq = op_ps.tile([128, 512], BF16, tag="op", name="tq")
                for i in range(4):
                    nc.tensor.transpose(tq[:, i * 128:(i + 1) * 128],
                                        vt_stage[:, i * 128:(i + 1) * 128],
                                        ident[:])
                ce(v_sb[h][:, sl], tq[:])

                ps_q = proj("q")
                ce(qt[h][:, sl], ps_q[:])
                ps_k = proj("k")
                ce(kt[h][:, sl], ps_k[:])

            # ---- attention core for one (head, strip) ----
            atn = [qkvp.tile([HD, S], BF16, tag=f"atn{h}", name=f"atn{h}")
                   for h in range(HEADS_PER_CORE)]

            def attn_core(h, s, l1eng):
                q_sl = slice(s * 512, (s + 1) * 512)
                at = at_ps.tile([HD, 512], F32, tag="at", name="at")
                tlev = []  # pair-sum tiles [128, 1024]
                for p in range(8):
                    scp = sc_ps.tile([128, 1024], F32, tag="sc", name="scp")
                    for i in range(2):
                        ktile = 2 * p + i
                        nc.tensor.matmul(
                            scp[:, i * 512:(i + 1) * 512],
                            kt[h][:, ktile * 128:(ktile + 1) * 128],
                            qt[h][:, q_sl],
                            start=True, stop=True, skip_group_check=True)
                    es_p = esp.tile([128, 1024], BF16, tag="es", name="es_p")
                    nc.scalar.activation(
                        es_p[:], scp[:], mybir.ActivationFunctionType.Exp,
                        scale=float(INV_SQRT_HD))
                    for i in range(2):
                        ktile = 2 * p + i
                        nc.tensor.matmul(
                            at[:],
                            v_sb[h][:, ktile * 128:(ktile + 1) * 128],
                            es_p[:, i * 512:(i + 1) * 512],
                            start=(ktile == 0), stop=(ktile == NKT - 1),
                            skip_group_check=True)
                    if p % 2 == 1:
                        t = spp.tile([128, 1024], BF16, tag="sp", name="tsum")
                        l1eng(t[:], prev_es[:], es_p[:])
                        tlev.append(t)
                    prev_es = es_p
                # tree levels 2/3 + halves-fold on DVE: 4 tiles -> [128, 512]
                nc.vector.tensor_add(tlev[0][:], tlev[0][:], tlev[1][:])
                nc.vector.tensor_add(tlev[2][:], tlev[2][:], tlev[3][:])
                nc.vector.tensor_add(tlev[0][:], tlev[0][:], tlev[2][:])
                spf = smallp.tile([128, 512], BF16, tag="spf", name="spf")
                nc.vector.tensor_add(spf[:], tlev[0][:, 0:512],
                                     tlev[0][:, 512:1024])
                # rank-1 all-ones matmul: partition-sum + broadcast in one op
                den_bc = op_ps.tile([128, 512], F32, tag="op", name="den_bc")
                nc.tensor.matmul(den_bc[:], ones_sq[:], spf[:],
                                 start=True, stop=True, skip_group_check=True)
                rc = smallp.tile([128, 512], F32, tag="rc", name="rc")
                nc.vector.reciprocal_approx_fast(rc[:], den_bc[:])
                nc.vector.tensor_mul(atn[h][:, q_sl], at[:], rc[:])

            # ---- output projection for one strip (both heads) ----
            def out_proj(s, scalar_assist=False):
                for j in range(4):
                    c_sl = slice(s * 512 + j * 128, s * 512 + (j + 1) * 128)
                    o_sb = outp.tile([128, D], BF16, tag="osb", name="o_sb")
                    for g in range(2):
                        ps = op_ps.tile([128, 512], F32, tag="op",
                                        name="opj_ps")
                        for h in range(HEADS_PER_CORE):
                            nc.tensor.matmul(
                                ps[:],
                                atn[h][:, c_sl],
                                wo_sb[h][:, g * 512:(g + 1) * 512],
                                start=(h == 0), stop=(h == 1),
                                skip_group_check=True)
                        dst = o_sb[:, g * 512:(g + 1) * 512]
                        if scalar_assist and j % 2 == 1:
                            nc.scalar.copy(dst, ps[:])
                        else:
                            nc.vector.tensor_copy(dst, ps[:])
                    r0 = s * 512 + j * 128
                    nc.sync.dma_start(out=out[r0:r0 + 128, :], in_=o_sb[:])

            # ---- schedule ----
            for s in range(NSTRIP):
                qkv_strip(0, s, nc.scalar.copy)
            add = nc.vector.tensor_add
            for s in range(NSTRIP):
                attn_core(0, s, add)
                qkv_strip(1, s, nc.vector.tensor_copy)
            attn_core(1, 0, add)
            attn_core(1, 1, add)
            out_proj(0, scalar_assist=True)
            attn_core(1, 2, add)
            out_proj(1, scalar_assist=True)
            attn_core(1, 3, add)
            out_proj(2, scalar_assist=True)
            emit_lora()
            out_proj(3, scalar_assist=True)

    nc.compile()
    _CACHE["nc"] = nc
    return nc


def _prep_in_maps(inputs):
    bf = ml_dtypes.bfloat16
    hidden = np.asarray(inputs["hidden_states"], dtype=np.float32)
    control = np.asarray(inputs["control_states"], dtype=np.float32)
    Wq = np.asarray(inputs["Wq"], dtype=np.float32)
    Wk = np.asarray(inputs["Wk"], dtype=np.float32)
    Wv = np.asarray(inputs["Wv"], dtype=np.float32)
    Wo = np.asarray(inputs["Wo"], dtype=np.float32)
    bo_in = np.ascontiguousarray(
        np.asarray(inputs["bo"], dtype=np.float32).reshape(1, D))
    ldT = np.ascontiguousarray(
        np.asarray(inputs["lora_down"], dtype=np.float32).T.astype(bf)
        .reshape(8, 128, RANK).transpose(1, 0, 2).reshape(128, 8 * RANK))
    luT = np.ascontiguousarray(
        np.asarray(inputs["lora_up"], dtype=np.float32).T.astype(bf))

    # sbuf-image shuffles: [D_chunk c, p, ...] -> partition-major rows
    def wshuf(wT):  # [1024, 256] -> [2*128, 8*128] (head-major rows)
        return np.ascontiguousarray(
            wT.reshape(8, 128, 2, HD).transpose(2, 1, 0, 3)
            .reshape(2 * 128, 8 * HD))

    xsh_b = []
    for b in range(B):
        xT = hidden[b].T.astype(bf)  # [1024, 2048]
        xsh_b.append(np.ascontiguousarray(
            xT.reshape(8, 128, NSTRIP, 512).transpose(2, 1, 0, 3)
            .reshape(NSTRIP * 128, 8 * 512)))
    cT_full = control.reshape(SG, D).T.astype(bf)

    in_maps = []
    for c in range(N_CORES):
        b = c // 4
        g = c % 4
        hs = slice(g * 256, (g + 1) * 256)
        rs = slice(c * ROWS_PER_CORE, (c + 1) * ROWS_PER_CORE)
        ct = cT_full[:, rs]  # [1024, 512]
        in_maps.append({
            "xsh": xsh_b[b],
            "wqT": wshuf(Wq[hs, :].T.astype(bf)),
            "wkT": wshuf(Wk[hs, :].T.astype(bf)),
            "wvT": wshuf(Wv[hs, :].T.astype(bf)),
            "woT": np.ascontiguousarray(Wo[:, hs].T.astype(bf)),
            "cT": np.ascontiguousarray(
                ct.reshape(8, 128, ROWS_PER_CORE).transpose(1, 0, 2)
                .reshape(128, 8 * ROWS_PER_CORE)),
            "ldT": ldT,
            "luT": luT,
            "bo": bo_in,
        })
    return in_maps


def _reduce_outputs(results):
    total = np.zeros((B, S, D), dtype=np.float32)
    for c in range(N_CORES):
        b = c // 4
        total[b] += results[c]["out"].astype(np.float32)
    flat = total.reshape(SG, D)
    for c in range(N_CORES):
        rs = slice(c * ROWS_PER_CORE, (c + 1) * ROWS_PER_CORE)
        lo = results[c]["lora_out"].astype(np.float32)
        flat[rs] += lo.reshape(128, 4, D).transpose(1, 0, 2).reshape(
            ROWS_PER_CORE, D)
    return flat.reshape(B, S, D)


def kernel(**inputs):
    nc = build_program()
    in_maps = _prep_in_maps(inputs)
    res = run_bass_kernel_spmd(nc, in_maps, list(range(N_CORES)))
    return _reduce_outputs(res.results)


# revision 37
# speedup vs baseline: 1.0347x; 1.0347x over previous
"""Trainium2 Bass kernel for ControlLoRACrossAttnProcessor (v2).

Batch x head-group sharding over 8 NeuronCores: core c owns batch c//4
and heads 2*(c%4), 2*(c%4)+1.  Each core computes both heads' full
attention over its batch and a partial output projection (contraction
over its 256 Wo rows); the host sums 4 partials per batch.  The rank-4
LoRA path is sharded by global sequence rows (core c owns rows
c*512:(c+1)*512) and emitted at the START of the program so it doubles
as PE warm-up; the output bias rides the LoRA partial (added exactly
once per row).

All matmuls are bf16 (hosts casts inputs); PSUM accumulates fp32.
Attention uses the transposed-scores layout: scores^T tiles
[k=128, q=512] so softmax's exp rides ScalarE.  Softmax denominators
avoid the PE entirely: a DVE tree-add collapses the 16 exp tiles per
(head, strip) to [128, 512], GpSimd partition_all_reduce produces the
per-q sums broadcast over all partitions, DVE reciprocal + tensor_mul
normalize the PV accumulator before the (per-q linear) output
projection.
"""

import sys

for _p in ("/opt/trn_rl_repo", "/root/.axon_site"):
    if _p not in sys.path:
        sys.path.insert(0, _p)

import numpy as np
import ml_dtypes

import concourse.bass as bass  # noqa: E402
import concourse.mybir as mybir  # noqa: E402
import concourse.bass_isa as bass_isa  # noqa: E402
from concourse import bacc  # noqa: E402
from concourse.bass_utils import run_bass_kernel_spmd  # noqa: E402
from concourse.tile import TileContext  # noqa: E402
from concourse.masks import make_identity  # noqa: E402

dt = mybir.dt

B, S, D = 2, 2048, 1024
H = 8
HD = 128
RANK = 4
N_CORES = 8
SG = B * S
ROWS_PER_CORE = SG // N_CORES      # 512 lora rows per core
NSTRIP = S // 512                  # 4 query strips of 512
NKT = S // 128                     # 16 key tiles of 128
HEADS_PER_CORE = 2
INV_SQRT_HD = 1.0 / np.sqrt(np.float32(HD))

F32 = dt.float32
BF16 = dt.bfloat16

_CACHE = {}


def build_program():
    if "nc" in _CACHE:
        return _CACHE["nc"]

    nc = bacc.Bacc("TRN2", target_bir_lowering=False, debug=False,
                   num_devices=N_CORES)

    # xsh/ct/w tensors are pre-shuffled on the host so each DMA is a plain
    # contiguous 2D copy (sbuf image layout), minimizing sync-engine
    # descriptor-generation time per dma_start.
    xsh = nc.declare_dram_parameter("xsh", [NSTRIP * 128, 8 * 512], BF16,
                                    isOutput=False)
    wqT = nc.declare_dram_parameter("wqT", [2 * 128, 8 * HD], BF16,
                                    isOutput=False)
    wkT = nc.declare_dram_parameter("wkT", [2 * 128, 8 * HD], BF16,
                                    isOutput=False)
    wvT = nc.declare_dram_parameter("wvT", [2 * 128, 8 * HD], BF16,
                                    isOutput=False)
    woT = nc.declare_dram_parameter("woT", [256, D], BF16, isOutput=False)
    cT = nc.declare_dram_parameter("cT", [128, 8 * ROWS_PER_CORE], BF16,
                                   isOutput=False)
    ldT = nc.declare_dram_parameter("ldT", [128, 8 * RANK], BF16,
                                    isOutput=False)
    luT = nc.declare_dram_parameter("luT", [RANK, D], BF16, isOutput=False)
    bo = nc.declare_dram_parameter("bo", [1, D], F32, isOutput=False)
    out = nc.declare_dram_parameter("out", [S, D], BF16, isOutput=True)
    lora_out = nc.declare_dram_parameter("lora_out", [128, 4 * D],
                                         BF16, isOutput=True)

    with TileContext(nc) as tc:
        with tc.tile_pool(name="const", bufs=1) as constp, \
             tc.tile_pool(name="wts", bufs=1) as wts, \
             tc.tile_pool(name="xs", bufs=1) as xsp, \
             tc.tile_pool(name="qkv", bufs=1) as qkvp, \
             tc.tile_pool(name="stage", bufs=2) as stagep, \
             tc.tile_pool(name="es", bufs=12) as esp, \
             tc.tile_pool(name="sp", bufs=8) as spp, \
             tc.tile_pool(name="small", bufs=2) as smallp, \
             tc.tile_pool(name="outp", bufs=2) as outp, \
             tc.tile_pool(name="sc_ps", bufs=2, space="PSUM") as sc_ps, \
             tc.tile_pool(name="at_ps", bufs=2, space="PSUM") as at_ps, \
             tc.tile_pool(name="op_ps", bufs=2, space="PSUM") as op_ps:

            # ---- constants ----
            ident = constp.tile([128, 128], BF16, tag="ident")
            make_identity(nc, ident[:])
            dummy = constp.tile([128, 512], BF16, tag="dummy")
            nc.vector.memset(dummy[:], 0.0)
            # rank-1 all-ones lhsT: one matmul = partition-sum + broadcast
            ones_sq = constp.tile([128, 128], BF16, tag="ones_sq")
            nc.vector.memset(ones_sq[:], 1.0)

            # ---- DMA loads, ordered for earliest compute start ----
            ld_sb = wts.tile([128, 8 * RANK], BF16, tag="ld")
            lu_sb = wts.tile([RANK, D], BF16, tag="lu")
            bo_sb = wts.tile([1, D], F32, tag="bo")
            bo_bc = wts.tile([128, D], F32, tag="bobc")
            ct_sb = wts.tile([128, 8 * ROWS_PER_CORE], BF16, tag="ct")
            nc.sync.dma_start(out=ld_sb[:], in_=ldT[:])
            nc.sync.dma_start(out=lu_sb[:], in_=luT[:])
            nc.sync.dma_start(out=bo_sb[:], in_=bo[:])

            w_sb = {}   # (which, head) -> [128, 8*128] tile
            for h in range(HEADS_PER_CORE):
                for nm in ("q", "k", "v"):
                    t = wts.tile([128, 8 * HD], BF16, tag=f"w{nm}{h}",
                                 name=f"w{nm}{h}")
                    w_sb[(nm, h)] = t

            xs = []
            for s in range(NSTRIP):
                t = xsp.tile([128, 8 * 512], BF16, tag=f"xs{s}",
                             name=f"xs{s}")
                xs.append(t)

            def load_xs(s):
                nc.sync.dma_start(out=xs[s][:],
                                  in_=xsh[s * 128:(s + 1) * 128, :])

            def load_w(h):
                for nm, src in (("q", wqT), ("k", wkT), ("v", wvT)):
                    nc.sync.dma_start(
                        out=w_sb[(nm, h)][:],
                        in_=src[h * 128:(h + 1) * 128, :])

            load_w(0)
            # strip 0 in two halves so the first proj can start on chunks 0-3
            nc.sync.dma_start(out=xs[0][:, 0:2048], in_=xsh[0:128, 0:2048])
            nc.sync.dma_start(out=xs[0][:, 2048:4096],
                              in_=xsh[0:128, 2048:4096])
            load_xs(1)
            nc.sync.dma_start(out=ct_sb[:], in_=cT[:])
            load_xs(2)
            load_xs(3)
            load_w(1)

            wo_sb = [wts.tile([HD, D], BF16, tag=f"wo{h}", name=f"wo{h}")
                     for h in range(HEADS_PER_CORE)]
            for h in range(HEADS_PER_CORE):
                nc.sync.dma_start(out=wo_sb[h][:],
                                  in_=woT[h * HD:(h + 1) * HD, :])

            # ---- PE warmup on the zero tile while first DMAs land ----
            for _wu in range(10):
                wu_ps = op_ps.tile([128, 512], F32, tag="op", name="wu_ps")
                nc.tensor.matmul(wu_ps[:], dummy[:, 0:128], dummy[:],
                                 start=True, stop=True)

            nc.gpsimd.partition_broadcast(bo_bc[:], bo_sb[:])

            # ---- LoRA path (also serves as further PE warmup) ----
            def emit_lora():
                dn_ps = op_ps.tile([128, ROWS_PER_CORE], F32, tag="op",
                                   name="dn_ps")
                for d in range(8):
                    nc.tensor.matmul(
                        dn_ps[0:RANK, :],
                        ld_sb[:, d * RANK:(d + 1) * RANK],
                        ct_sb[:, d * ROWS_PER_CORE:(d + 1) * ROWS_PER_CORE],
                        start=(d == 0), stop=(d == 7))
                dn_sb = smallp.tile([RANK, ROWS_PER_CORE], BF16, tag="dn",
                                    name="dn_sb")
                nc.scalar.copy(dn_sb[:], dn_ps[0:RANK, :])
                lo_sb = outp.tile([128, 4 * D], BF16, tag="lo", bufs=1,
                                  name="lo_sb")
                for j in range(ROWS_PER_CORE // 128):
                    for g in range(2):
                        up_ps = op_ps.tile([128, 512], F32, tag="op",
                                           name="up_ps")
                        nc.tensor.matmul(
                            up_ps[:],
                            dn_sb[:, j * 128:(j + 1) * 128],
                            lu_sb[:, g * 512:(g + 1) * 512],
                            start=True, stop=True)
                        nc.vector.tensor_add(
                            lo_sb[:, j * D + g * 512:j * D + (g + 1) * 512],
                            up_ps[:], bo_bc[:, g * 512:(g + 1) * 512])
                nc.sync.dma_start(out=lora_out[:], in_=lo_sb[:])

            # ---- QKV projection for one (head, strip) ----
            qt = [qkvp.tile([HD, S], BF16, tag=f"qt{h}", name=f"qt{h}")
                  for h in range(HEADS_PER_CORE)]
            kt = [qkvp.tile([HD, S], BF16, tag=f"kt{h}", name=f"kt{h}")
                  for h in range(HEADS_PER_CORE)]
            v_sb = [qkvp.tile([128, S], BF16, tag=f"v{h}", name=f"v{h}")
                    for h in range(HEADS_PER_CORE)]

            def qkv_strip(h, s, ce):
                sl = slice(s * 512, (s + 1) * 512)

                def proj(nm):
                    ps = op_ps.tile([128, 512], F32, tag="op", name="pj_ps")
                    for d in range(8):
                        nc.tensor.matmul(
                            ps[:],
                            w_sb[(nm, h)][:, d * HD:(d + 1) * HD],
                            xs[s][:, d * 512:(d + 1) * 512],
                            start=(d == 0), stop=(d == 7),
                            skip_group_check=True)
                    return ps

                ps_v = proj("v")
                vt_stage = stagep.tile([HD, 512], BF16, tag="vst",
                                       name="vt_stage")
                ce(vt_stage[:], ps_v[:])
                tq = op_ps.tile([128, 512], BF16, tag="op", name="tq")
                for i in range(4):
                    nc.tensor.transpose(tq[:, i * 128:(i + 1) * 128],
                                        vt_stage[:, i * 128:(i + 1) * 128],
                                        ident[:])
                ce(v_sb[h][:, sl], tq[:])

                ps_q = proj("q")
                ce(qt[h][:, sl], ps_q[:])
                ps_k = proj("k")
                ce(kt[h][:, sl], ps_k[:])

            # ---- attention core for one (head, strip) ----
            atn = [qkvp.tile([HD, S], BF16, tag=f"atn{h}", name=f"atn{h}")
                   for h in range(HEADS_PER_CORE)]

            def attn_core(h, s, l1eng):
                q_sl = slice(s * 512, (s + 1) * 512)
                at = at_ps.tile([HD, 512], F32, tag="at", name="at")
                tlev = []  # pair-sum tiles [128, 1024]
                for p in range(8):
                    scp = sc_ps.tile([128, 1024], F32, tag="sc", name="scp")
                    for i in range(2):
                        ktile = 2 * p + i
                        nc.tensor.matmul(
                            scp[:, i * 512:(i + 1) * 512],
                            kt[h][:, ktile * 128:(ktile + 1) * 128],
                            qt[h][:, q_sl],
                            start=True, stop=True, skip_group_check=True)
                    es_p = esp.tile([128, 1024], BF16, tag="es", name="es_p")
                    nc.scalar.activation(
                        es_p[:], scp[:], mybir.ActivationFunctionType.Exp,
                        scale=float(INV_SQRT_HD))
                    for i in range(2):
                        ktile = 2 * p + i
                        nc.tensor.matmul(
                            at[:],
                            v_sb[h][:, ktile * 128:(ktile + 1) * 128],
                            es_p[:, i * 512:(i + 1) * 512],
                            start=(ktile == 0), stop=(ktile == NKT - 1),
                            skip_group_check=True)
                    if p % 2 == 1:
                        t = spp.tile([128, 1024], BF16, tag="sp", name="tsum")
                        l1eng(t[:], prev_es[:], es_p[:])
                        tlev.append(t)
                    prev_es = es_p
                # tree levels 2/3 + halves-fold on DVE: 4 tiles -> [128, 512]
                nc.vector.tensor_add(tlev[0][:], tlev[0][:], tlev[1][:])
                nc.vector.tensor_add(tlev[2][:], tlev[2][:], tlev[3][:])
                nc.vector.tensor_add(tlev[0][:], tlev[0][:], tlev[2][:])
                spf = smallp.tile([128, 512], BF16, tag="spf", name="spf")
                nc.vector.tensor_add(spf[:], tlev[0][:, 0:512],
                                     tlev[0][:, 512:1024])
                # rank-1 all-ones matmul: partition-sum + broadcast in one op
                den_bc = op_ps.tile([128, 512], F32, tag="op", name="den_bc")
                nc.tensor.matmul(den_bc[:], ones_sq[:], spf[:],
                                 start=True, stop=True, skip_group_check=True)
                rc = smallp.tile([128, 512], F32, tag="rc", name="rc")
                nc.vector.reciprocal_approx_fast(rc[:], den_bc[:])
                nc.vector.tensor_mul(atn[h][:, q_sl], at[:], rc[:])

            # ---- output projection for one strip (both heads) ----
            def out_proj(s, scalar_assist=False):
                for j in range(4):
                    c_sl = slice(s * 512 + j * 128, s * 512 + (j + 1) * 128)
                    o_sb = outp.tile([128, D], BF16, tag="osb", name="o_sb")
                    for g in range(2):
                        ps = op_ps.tile([128, 512], F32, tag="op",
                                        name="opj_ps")
                        for h in range(HEADS_PER_CORE):
                            nc.tensor.matmul(
                                ps[:],
                                atn[h][:, c_sl],
                                wo_sb[h][:, g * 512:(g + 1) * 512],
                                start=(h == 0), stop=(h == 1),
                                skip_group_check=True)
                        dst = o_sb[:, g * 512:(g + 1) * 512]
                        if scalar_assist and j % 2 == 1:
                            nc.scalar.copy(dst, ps[:])
                        else:
                            nc.vector.tensor_copy(dst, ps[:])
                    r0 = s * 512 + j * 128
                    nc.sync.dma_start(out=out[r0:r0 + 128, :], in_=o_sb[:])

            # ---- schedule ----
            for s in range(NSTRIP):
                qkv_strip(0, s, nc.scalar.copy)
            emit_lora()
            add = nc.vector.tensor_add
            for s in range(NSTRIP):
                attn_core(0, s, add)
                qkv_strip(1, s, nc.vector.tensor_copy)
            attn_core(1, 0, add)
            attn_core(1, 1, add)
            out_proj(0)
            attn_core(1, 2, add)
            out_proj(1)
            attn_core(1, 3, add)
            out_proj(2, scalar_assist=True)
            out_proj(3, scalar_assist=True)

    nc.compile()
    _CACHE["nc"] = nc
    return nc


def _prep_in_maps(inputs):
    bf = ml_dtypes.bfloat16
    hidden = np.asarray(inputs["hidden_states"], dtype=np.float32)
    control = np.asarray(inputs["control_states"], dtype=np.float32)
    Wq = np.asarray(inputs["Wq"], dtype=np.float32)
    Wk = np.asarray(inputs["Wk"], dtype=np.float32)
    Wv = np.asarray(inputs["Wv"], dtype=np.float32)
    Wo = np.asarray(inputs["Wo"], dtype=np.float32)
    bo_in = np.ascontiguousarray(
        np.asarray(inputs["bo"], dtype=np.float32).reshape(1, D))
    ldT = np.ascontiguousarray(
        np.asarray(inputs["lora_down"], dtype=np.float32).T.astype(bf)
        .reshape(8, 128, RANK).transpose(1, 0, 2).reshape(128, 8 * RANK))
    luT = np.ascontiguousarray(
        np.asarray(inputs["lora_up"], dtype=np.float32).T.astype(bf))

    # sbuf-image shuffles: [D_chunk c, p, ...] -> partition-major rows
    def wshuf(wT):  # [1024, 256] -> [2*128, 8*128] (head-major rows)
        return np.ascontiguousarray(
            wT.reshape(8, 128, 2, HD).transpose(2, 1, 0, 3)
            .reshape(2 * 128, 8 * HD))

    xsh_b = []
    for b in range(B):
        xT = hidden[b].T.astype(bf)  # [1024, 2048]
        xsh_b.append(np.ascontiguousarray(
            xT.reshape(8, 128, NSTRIP, 512).transpose(2, 1, 0, 3)
            .reshape(NSTRIP * 128, 8 * 512)))
    cT_full = control.reshape(SG, D).T.astype(bf)

    in_maps = []
    for c in range(N_CORES):
        b = c // 4
        g = c % 4
        hs = slice(g * 256, (g + 1) * 256)
        rs = slice(c * ROWS_PER_CORE, (c + 1) * ROWS_PER_CORE)
        ct = cT_full[:, rs]  # [1024, 512]
        in_maps.append({
            "xsh": xsh_b[b],
            "wqT": wshuf(Wq[hs, :].T.astype(bf)),
            "wkT": wshuf(Wk[hs, :].T.astype(bf)),
            "wvT": wshuf(Wv[hs, :].T.astype(bf)),
            "woT": np.ascontiguousarray(Wo[:, hs].T.astype(bf)),
            "cT": np.ascontiguousarray(
                ct.reshape(8, 128, ROWS_PER_CORE).transpose(1, 0, 2)
                .reshape(128, 8 * ROWS_PER_CORE)),
            "ldT": ldT,
            "luT": luT,
            "bo": bo_in,
        })
    return in_maps


def _reduce_outputs(results):
    total = np.zeros((B, S, D), dtype=np.float32)
    for c in range(N_CORES):
        b = c // 4
        total[b] += results[c]["out"].astype(np.float32)
    flat = total.reshape(SG, D)
    for c in range(N_CORES):
        rs = slice(c * ROWS_PER_CORE, (c + 1) * ROWS_PER_CORE)
        lo = results[c]["lora_out"].astype(np.float32)
        flat[rs] += lo.reshape(128, 4, D).transpose(1, 0, 2).reshape(
            ROWS_PER_CORE, D)
    return flat.reshape(B, S, D)


def kernel(**inputs):
    nc = build_program()
    in_maps = _prep_in_maps(inputs)
    res = run_bass_kernel_spmd(nc, in_maps, list(range(N_CORES)))
    return _reduce_outputs(res.results)
